# revision 14
# baseline (speedup 1.0000x reference)
"""Multi-head attention Bass/Tile kernel for Trainium2.

Full inputs: q,k,v [8, 16, 1024, 128] fp32. Shards batch across 8 cores.
Per core/head: scores^T = (K @ Q^T)/128 via PE (bf16), exp split between
ACT (hw exp) and DVE (custom quartic-approx uop), PV with P^T stationary
and V||ones moving so the softmax denominator falls out of the same
matmul pass; normalize on DVE.
"""

import os
from contextlib import ExitStack

import numpy as np

import concourse.bass as bass
import concourse.tile as tile
from concourse import bacc, dve_ops, mybir
from concourse.bass_utils import run_bass_kernel_spmd
from concourse.dve_spec import C0, C1, C2, One, Spec, Src0, Src1, lower, sq
from concourse.dve_spec import _has_src1 as has_src1
from concourse.dve_table_gen import dve_ver_for
from concourse.dve_uop import DveOpSpec

H, S, D = 16, 1024, 128
NB = S // 128  # 8 blocks of 128 along sequence
FP32 = mybir.dt.float32
BF16 = mybir.dt.bfloat16
AF = mybir.ActivationFunctionType

# exp(t) ~= (sq(a*t+b)+c) * sq(s*t+1), max rel err 5.5e-4 on |t|<=0.62
# (raw scores here are <= ~0.55 after the 1/128 scale, folded into a and s).
EA, EB, EC, ES = 0.42048895, 0.30027227, 0.90964238, 0.37396779
# Number of score tiles per head whose exp runs on DVE instead of ACT.
DVE_EXP_IBS = int(os.environ.get("DVE_EXP_IBS", "1"))


def _register_exp_op():
    name = "EXP_QUARTIC_ANT"
    for op in dve_ops.OPS:
        if op.name == name:
            return op
    body = (sq(Src0 * C0 + C1) + C2) * sq(Src0 * Src1 + One)

    def ref(in0, in1, s0, s1, imm2):
        x = in0.astype(np.float32)
        return (np.square(x * s0 + s1) + imm2) * np.square(x * in1 + 1.0)

    spec = Spec(body=body, reference=ref)
    row = dve_ops._CUSTOM_DVE_ROW_BASE + len(dve_ops.OPS)
    shas = {}
    for ver in ("v3",):
        tmp = DveOpSpec(
            name=name, opcode=row, uops=lower(spec, ver=ver), rd1_en=has_src1(spec)
        )
        shas[ver] = tmp.sha(ver)
    op = dve_ops.DveOp(name, spec, subdim=False, uops_sha=shas)
    dve_ops.OPS.append(op)
    dve_ops._SUB_OPCODE_FOR_NAME[name] = row
    dve_ops.CUSTOM_DVE_SPECS[name] = spec
    return op


def _emit_head_prep(nc, tc, pools, aps, h):
    """Loads + transposes for head h. Returns (qT, kT, va)."""
    (ld_pool, tq_pool, v_pool, pt_pool, _out_pool, _small_pool,
     ps_s, _ps_o, dconst) = pools
    q, k, v, _out = aps

    # Natural-layout loads with fp32->bf16 cast during DMA (SWDGE).
    qn = ld_pool.tile([128, S], BF16, tag="qn")
    kn = ld_pool.tile([128, S], BF16, tag="kn")
    nc.gpsimd.dma_start(
        out=qn[:].rearrange("p (sb d) -> p sb d", d=D),
        in_=q[h].rearrange("(sb p) d -> p sb d", p=128),
    )
    nc.gpsimd.dma_start(
        out=kn[:].rearrange("p (sb d) -> p sb d", d=D),
        in_=k[h].rearrange("(sb p) d -> p sb d", p=128),
    )

    # V augmented with a ones column: [128, NB*(D+1)] bf16.
    va = v_pool.tile([128, NB * (D + 1)], BF16, tag="va")
    va3 = va[:].rearrange("p (ib e) -> p ib e", e=D + 1)
    nc.gpsimd.dma_start(
        out=va3[:, :, 0:D],
        in_=v[h].rearrange("(ib p) d -> p ib d", p=128),
    )
    nc.gpsimd.memset(va3[:, :, D : D + 1], 1.0)

    # Whole-tile blocked xbar DMA transpose: one call per tensor gives
    # qT[d, sb*128+s'] = qn[s', sb*128+d], i.e. [d, s] layout per s-block.
    qT = tq_pool.tile([128, S], BF16, tag="qT")
    kT = tq_pool.tile([128, S], BF16, tag="kT")
    for src, dst in ((qn, qT), (kn, kT)):
        nc.sync.dma_start(
            out=dst[:].rearrange("p (e l) -> p e l", l=128),
            in_=src[:],
            transpose=True,
        )

    return qT, kT, va


def _emit_qk_exp(nc, pools, exp_op, qT, kT, ib):
    """One i-block of QK^T + exp; returns the P^T tile."""
    (_ld, _tq, _v, pt_pool, _out, _small, ps_s, _ps_o, dconst) = pools
    ps = ps_s.tile([128, S], FP32)
    for jh in range(2):
        nc.tensor.matmul(
            ps[:, jh * 512 : (jh + 1) * 512],
            kT[:, ib * 128 : (ib + 1) * 128],
            qT[:, jh * 512 : (jh + 1) * 512],
            start=True,
            stop=True,
        )
    ptile = pt_pool.tile([128, S], BF16, tag=f"pt{ib}")
    if ib < NB - DVE_EXP_IBS:
        nc.scalar.activation(ptile[:], ps[:], AF.Exp, scale=1.0 / D)
    else:
        nc.vector._custom_dve(
            exp_op, out=ptile[:], in0=ps[:], in1=dconst[:],
            s0=EA / D, s1=EB, imm2=EC,
        )
    return ptile


def _emit_pv_norm(nc, pools, ptiles, va, ob, jb):
    """One j-block of PV + normalize into ob."""
    (_ld, _tq, _v, _pt, _out, small_pool, _ps_s, ps_o, _dc) = pools
    va3 = va[:].rearrange("p (ib e) -> p ib e", e=D + 1)
    po = ps_o.tile([128, D + 1], FP32)
    for ib in range(NB):
        nc.tensor.matmul(
            po[:],
            ptiles[ib][:, jb * 128 : (jb + 1) * 128],
            va3[:, ib, :],
            start=(ib == 0),
            stop=(ib == NB - 1),
        )
    rec = small_pool.tile([128, 1], FP32, tag="rec")
    nc.vector.reciprocal(rec[:], po[:, D : D + 1])
    nc.vector.tensor_scalar_mul(
        ob[:, jb * 128 : (jb + 1) * 128], po[:, 0:D], rec[:]
    )


def _emit_store(nc, pools, aps, h, ob):
    out = aps[3]
    nc.scalar.dma_start(
        out=out[h].rearrange("(jb p) d -> p jb d", p=128),
        in_=ob[:].rearrange("p (jb d) -> p jb d", d=D),
    )


def build_bass():
    exp_op = _register_exp_op()
    nc = bacc.Bacc("TRN2", target_bir_lowering=False, debug=False)
    q = nc.dram_tensor("q", [H, S, D], FP32, kind="ExternalInput").ap()
    k = nc.dram_tensor("k", [H, S, D], FP32, kind="ExternalInput").ap()
    v = nc.dram_tensor("v", [H, S, D], FP32, kind="ExternalInput").ap()
    out = nc.dram_tensor("out", [H, S, D], FP32, kind="ExternalOutput").ap()
    aps = (q, k, v, out)

    with ExitStack() as ctx:
        tc = ctx.enter_context(tile.TileContext(nc))
        const_pool = ctx.enter_context(tc.tile_pool(name="const", bufs=1))
        dconst = const_pool.tile([128, S], FP32)
        nc.gpsimd.memset(dconst[:], ES / D)

        ld_pool = ctx.enter_context(tc.tile_pool(name="loads", bufs=4))
        tq_pool = ctx.enter_context(tc.tile_pool(name="qkT", bufs=4))
        v_pool = ctx.enter_context(tc.tile_pool(name="vaug", bufs=4))
        pt_pool = ctx.enter_context(tc.tile_pool(name="pT", bufs=2))
        out_pool = ctx.enter_context(tc.tile_pool(name="outs", bufs=2))
        small_pool = ctx.enter_context(tc.tile_pool(name="small", bufs=4))
        ps_s = ctx.enter_context(tc.tile_pool(name="ps_s", bufs=3, space="PSUM"))
        ps_o = ctx.enter_context(tc.tile_pool(name="ps_o", bufs=2, space="PSUM"))
        pools = (ld_pool, tq_pool, v_pool, pt_pool, out_pool, small_pool,
                 ps_s, ps_o, dconst)

        # Software pipeline, block-interleaved: QK/exp of head h alternate
        # with PV/normalize of head h-1 so the PE always has ready work.
        out_pool = pools[4]
        prev = None  # (ptiles, va) of head h-1
        for h in range(H + 1):
            if h < H:
                qT, kT, va = _emit_head_prep(nc, tc, pools, aps, h)
                ptiles = []
            if prev is not None:
                ob = out_pool.tile([128, S], FP32, tag="ob")
            for x in range(NB):
                if h < H:
                    ptiles.append(_emit_qk_exp(nc, pools, exp_op, qT, kT, x))
                if prev is not None:
                    _emit_pv_norm(nc, pools, prev[0], prev[1], ob, x)
            if prev is not None:
                _emit_store(nc, pools, aps, h - 1, ob)
            prev = (ptiles, va) if h < H else None
    nc.finalize()
    return nc


_NC_CACHE = None


def _get_nc():
    global _NC_CACHE
    if _NC_CACHE is None:
        _NC_CACHE = build_bass()
    return _NC_CACHE


def run_sharded(q, k, v, **kwargs):
    """q,k,v: full [8, 16, 1024, 128] fp32. Returns (results, BassKernelResults)."""
    B = q.shape[0]
    nc = _get_nc()
    in_maps = [
        {
            "q": np.ascontiguousarray(q[c], dtype=np.float32),
            "k": np.ascontiguousarray(k[c], dtype=np.float32),
            "v": np.ascontiguousarray(v[c], dtype=np.float32),
        }
        for c in range(B)
    ]
    res = run_bass_kernel_spmd(nc, in_maps, core_ids=list(range(B)), **kwargs)
    out = np.stack([res.results[c]["out"] for c in range(B)]).astype(np.float32)
    return out, res


def kernel(q, k, v):
    q = np.asarray(q)
    k = np.asarray(k)
    v = np.asarray(v)
    out, _ = run_sharded(q, k, v)
    return out


if __name__ == "__main__":
    rng = np.random.default_rng(0)
    q = rng.standard_normal((8, H, S, D), dtype=np.float32)
    k = rng.standard_normal((8, H, S, D), dtype=np.float32)
    v = rng.standard_normal((8, H, S, D), dtype=np.float32)
    o = kernel(q, k, v)
    print("out", o.shape, o.dtype, float(np.abs(o).mean()))


# revision 15
# speedup vs baseline: 1.8033x; 1.8033x over previous
"""Multi-head attention Bass/Tile kernel for Trainium2.

Full inputs: q,k,v [8, 16, 1024, 128] fp32. Shards batch across 8 cores.
Per core/head: scores^T = (K @ Q^T)/128 via PE (bf16), exp split between
ACT (hw exp) and DVE (custom quartic-approx uop), PV with P^T stationary
and V||ones moving so the softmax denominator falls out of the same
matmul pass; normalize on DVE.
"""

import os
from contextlib import ExitStack

import numpy as np

import concourse.bass as bass
import concourse.tile as tile
from concourse.masks import make_identity
from concourse import bacc, dve_ops, mybir
from concourse.bass_utils import run_bass_kernel_spmd
from concourse.dve_spec import C0, C1, C2, One, Spec, Src0, Src1, lower, sq
from concourse.dve_spec import _has_src1 as has_src1
from concourse.dve_table_gen import dve_ver_for
from concourse.dve_uop import DveOpSpec

H, S, D = 16, 1024, 128
NB = S // 128  # 8 blocks of 128 along sequence
FP32 = mybir.dt.float32
BF16 = mybir.dt.bfloat16
AF = mybir.ActivationFunctionType

# exp(t) ~= (sq(a*t+b)+c) * sq(s*t+1), max rel err 5.5e-4 on |t|<=0.62
# (raw scores here are <= ~0.55 after the 1/128 scale, folded into a and s).
EA, EB, EC, ES = 0.42048895, 0.30027227, 0.90964238, 0.37396779
# Number of score tiles per head whose exp runs on DVE instead of ACT.
DVE_EXP_IBS = int(os.environ.get("DVE_EXP_IBS", "1"))


def _register_exp_op():
    name = "EXP_QUARTIC_ANT"
    for op in dve_ops.OPS:
        if op.name == name:
            return op
    body = (sq(Src0 * C0 + C1) + C2) * sq(Src0 * Src1 + One)

    def ref(in0, in1, s0, s1, imm2):
        x = in0.astype(np.float32)
        return (np.square(x * s0 + s1) + imm2) * np.square(x * in1 + 1.0)

    spec = Spec(body=body, reference=ref)
    row = dve_ops._CUSTOM_DVE_ROW_BASE + len(dve_ops.OPS)
    shas = {}
    for ver in ("v3",):
        tmp = DveOpSpec(
            name=name, opcode=row, uops=lower(spec, ver=ver), rd1_en=has_src1(spec)
        )
        shas[ver] = tmp.sha(ver)
    op = dve_ops.DveOp(name, spec, subdim=False, uops_sha=shas)
    dve_ops.OPS.append(op)
    dve_ops._SUB_OPCODE_FOR_NAME[name] = row
    dve_ops.CUSTOM_DVE_SPECS[name] = spec
    return op


def _emit_head_prep(nc, tc, pools, aps, h):
    """Loads + transposes for head h. Returns (qT, kT, va)."""
    (ld_pool, tq_pool, v_pool, pt_pool, _out_pool, _small_pool,
     ps_t, ps_s, _ps_o, ident, dconst) = pools
    q, k, v, _out = aps

    # Natural-layout loads with fp32->bf16 cast during DMA (SWDGE).
    qn = ld_pool.tile([128, S], BF16, tag="qn")
    kn = ld_pool.tile([128, S], BF16, tag="kn")
    nc.gpsimd.dma_start(
        out=qn[:].rearrange("p (sb d) -> p sb d", d=D),
        in_=q[h].rearrange("(sb p) d -> p sb d", p=128),
    )
    nc.gpsimd.dma_start(
        out=kn[:].rearrange("p (sb d) -> p sb d", d=D),
        in_=k[h].rearrange("(sb p) d -> p sb d", p=128),
    )

    # V augmented with a ones column: [128, NB*(D+1)] bf16.
    va = v_pool.tile([128, NB * (D + 1)], BF16, tag="va")
    va3 = va[:].rearrange("p (ib e) -> p ib e", e=D + 1)
    nc.gpsimd.dma_start(
        out=va3[:, :, 0:D],
        in_=v[h].rearrange("(ib p) d -> p ib d", p=128),
    )
    nc.gpsimd.memset(va3[:, :, D : D + 1], 1.0)

    # Transposes as normal-mode matmuls (block stationary, identity moving):
    # out[d, s] = blk[s, d].T @ I. These pipeline like regular matmuls.
    qT = tq_pool.tile([128, S], BF16, tag="qT")
    kT = tq_pool.tile([128, S], BF16, tag="kT")
    for src, dst in ((qn, qT), (kn, kT)):
        for half in range(2):
            pth = ps_t.tile([128, 512], FP32)
            for g in range(4):
                sb = half * 4 + g
                nc.tensor.matmul(
                    pth[:, g * 128 : (g + 1) * 128],
                    src[:, sb * 128 : (sb + 1) * 128],
                    ident[:],
                    start=True,
                    stop=True,
                )
            nc.vector.tensor_copy(dst[:, half * 512 : (half + 1) * 512], pth[:])

    return qT, kT, va


def _emit_qk_exp(nc, pools, exp_op, qT, kT, ib):
    """One i-block of QK^T + exp; returns the P^T tile."""
    (_ld, _tq, _v, pt_pool, _out, _small, _ps_t, ps_s, _ps_o, _id, dconst) = pools
    ps = ps_s.tile([128, S], FP32)
    for jh in range(2):
        nc.tensor.matmul(
            ps[:, jh * 512 : (jh + 1) * 512],
            kT[:, ib * 128 : (ib + 1) * 128],
            qT[:, jh * 512 : (jh + 1) * 512],
            start=True,
            stop=True,
        )
    ptile = pt_pool.tile([128, S], BF16, tag=f"pt{ib}")
    if ib < NB - DVE_EXP_IBS:
        nc.scalar.activation(ptile[:], ps[:], AF.Exp, scale=1.0 / D)
    else:
        nc.vector._custom_dve(
            exp_op, out=ptile[:], in0=ps[:], in1=dconst[:],
            s0=EA / D, s1=EB, imm2=EC,
        )
    return ptile


def _emit_pv_norm(nc, pools, ptiles, va, ob, jb):
    """One j-block of PV + normalize into ob."""
    (_ld, _tq, _v, _pt, _out, small_pool, _ps_t, _ps_s, ps_o, _id, _dc) = pools
    va3 = va[:].rearrange("p (ib e) -> p ib e", e=D + 1)
    po = ps_o.tile([128, D + 1], FP32)
    for ib in range(NB):
        nc.tensor.matmul(
            po[:],
            ptiles[ib][:, jb * 128 : (jb + 1) * 128],
            va3[:, ib, :],
            start=(ib == 0),
            stop=(ib == NB - 1),
        )
    rec = small_pool.tile([128, 1], FP32, tag="rec")
    nc.vector.reciprocal(rec[:], po[:, D : D + 1])
    nc.vector.tensor_scalar_mul(
        ob[:, jb * 128 : (jb + 1) * 128], po[:, 0:D], rec[:]
    )


def _emit_store(nc, pools, aps, h, ob):
    out = aps[3]
    nc.scalar.dma_start(
        out=out[h].rearrange("(jb p) d -> p jb d", p=128),
        in_=ob[:].rearrange("p (jb d) -> p jb d", d=D),
    )


def build_bass():
    exp_op = _register_exp_op()
    nc = bacc.Bacc("TRN2", target_bir_lowering=False, debug=False)
    q = nc.dram_tensor("q", [H, S, D], FP32, kind="ExternalInput").ap()
    k = nc.dram_tensor("k", [H, S, D], FP32, kind="ExternalInput").ap()
    v = nc.dram_tensor("v", [H, S, D], FP32, kind="ExternalInput").ap()
    out = nc.dram_tensor("out", [H, S, D], FP32, kind="ExternalOutput").ap()
    aps = (q, k, v, out)

    with ExitStack() as ctx:
        tc = ctx.enter_context(tile.TileContext(nc))
        const_pool = ctx.enter_context(tc.tile_pool(name="const", bufs=1))
        ident = const_pool.tile([128, 128], BF16)
        make_identity(nc, ident[:])
        dconst = const_pool.tile([128, S], FP32)
        nc.gpsimd.memset(dconst[:], ES / D)

        ld_pool = ctx.enter_context(tc.tile_pool(name="loads", bufs=4))
        tq_pool = ctx.enter_context(tc.tile_pool(name="qkT", bufs=4))
        v_pool = ctx.enter_context(tc.tile_pool(name="vaug", bufs=4))
        pt_pool = ctx.enter_context(tc.tile_pool(name="pT", bufs=2))
        out_pool = ctx.enter_context(tc.tile_pool(name="outs", bufs=2))
        small_pool = ctx.enter_context(tc.tile_pool(name="small", bufs=4))
        ps_t = ctx.enter_context(tc.tile_pool(name="ps_t", bufs=2, space="PSUM"))
        ps_s = ctx.enter_context(tc.tile_pool(name="ps_s", bufs=2, space="PSUM"))
        ps_o = ctx.enter_context(tc.tile_pool(name="ps_o", bufs=2, space="PSUM"))
        pools = (ld_pool, tq_pool, v_pool, pt_pool, out_pool, small_pool,
                 ps_t, ps_s, ps_o, ident, dconst)

        # Software pipeline, block-interleaved: QK/exp of head h alternate
        # with PV/normalize of head h-1 so the PE always has ready work.
        out_pool = pools[4]
        prev = None  # (ptiles, va) of head h-1
        for h in range(H + 1):
            if h < H:
                qT, kT, va = _emit_head_prep(nc, tc, pools, aps, h)
                ptiles = []
            if prev is not None:
                ob = out_pool.tile([128, S], FP32, tag="ob")
            for x in range(NB):
                if h < H:
                    ptiles.append(_emit_qk_exp(nc, pools, exp_op, qT, kT, x))
                if prev is not None:
                    _emit_pv_norm(nc, pools, prev[0], prev[1], ob, x)
            if prev is not None:
                _emit_store(nc, pools, aps, h - 1, ob)
            prev = (ptiles, va) if h < H else None
    nc.finalize()
    return nc


_NC_CACHE = None


def _get_nc():
    global _NC_CACHE
    if _NC_CACHE is None:
        _NC_CACHE = build_bass()
    return _NC_CACHE


def run_sharded(q, k, v, **kwargs):
    """q,k,v: full [8, 16, 1024, 128] fp32. Returns (results, BassKernelResults)."""
    B = q.shape[0]
    nc = _get_nc()
    in_maps = [
        {
            "q": np.ascontiguousarray(q[c], dtype=np.float32),
            "k": np.ascontiguousarray(k[c], dtype=np.float32),
            "v": np.ascontiguousarray(v[c], dtype=np.float32),
        }
        for c in range(B)
    ]
    res = run_bass_kernel_spmd(nc, in_maps, core_ids=list(range(B)), **kwargs)
    out = np.stack([res.results[c]["out"] for c in range(B)]).astype(np.float32)
    return out, res


def kernel(q, k, v):
    q = np.asarray(q)
    k = np.asarray(k)
    v = np.asarray(v)
    out, _ = run_sharded(q, k, v)
    return out


if __name__ == "__main__":
    rng = np.random.default_rng(0)
    q = rng.standard_normal((8, H, S, D), dtype=np.float32)
    k = rng.standard_normal((8, H, S, D), dtype=np.float32)
    v = rng.standard_normal((8, H, S, D), dtype=np.float32)
    o = kernel(q, k, v)
    print("out", o.shape, o.dtype, float(np.abs(o).mean()))


# revision 16
# speedup vs baseline: 1.8799x; 1.0425x over previous
"""Multi-head attention Bass/Tile kernel for Trainium2.

Full inputs: q,k,v [8, 16, 1024, 128] fp32. Shards batch across 8 cores.
Per core/head: scores^T = (K @ Q^T)/128 via PE (bf16), exp split between
ACT (hw exp) and DVE (custom quartic-approx uop), PV with P^T stationary
and V||ones moving so the softmax denominator falls out of the same
matmul pass; normalize on DVE.
"""

import os
from contextlib import ExitStack

import numpy as np

import concourse.bass as bass
import concourse.tile as tile
from concourse.masks import make_identity
from concourse import bacc, dve_ops, mybir
from concourse.bass_utils import run_bass_kernel_spmd
from concourse.dve_spec import C0, C1, C2, One, Spec, Src0, Src1, lower, sq
from concourse.dve_spec import _has_src1 as has_src1
from concourse.dve_table_gen import dve_ver_for
from concourse.dve_uop import DveOpSpec

H, S, D = 16, 1024, 128
NB = S // 128  # 8 blocks of 128 along sequence
FP32 = mybir.dt.float32
BF16 = mybir.dt.bfloat16
AF = mybir.ActivationFunctionType

# exp(t) ~= (sq(a*t+b)+c) * sq(s*t+1), max rel err 5.5e-4 on |t|<=0.62
# (raw scores here are <= ~0.55 after the 1/128 scale, folded into a and s).
EA, EB, EC, ES = 0.42048895, 0.30027227, 0.90964238, 0.37396779
# Number of score tiles per head whose exp runs on DVE instead of ACT.
DVE_EXP_IBS = int(os.environ.get("DVE_EXP_IBS", "1"))


def _register_exp_op():
    name = "EXP_QUARTIC_ANT"
    for op in dve_ops.OPS:
        if op.name == name:
            return op
    body = (sq(Src0 * C0 + C1) + C2) * sq(Src0 * Src1 + One)

    def ref(in0, in1, s0, s1, imm2):
        x = in0.astype(np.float32)
        return (np.square(x * s0 + s1) + imm2) * np.square(x * in1 + 1.0)

    spec = Spec(body=body, reference=ref)
    row = dve_ops._CUSTOM_DVE_ROW_BASE + len(dve_ops.OPS)
    shas = {}
    for ver in ("v3",):
        tmp = DveOpSpec(
            name=name, opcode=row, uops=lower(spec, ver=ver), rd1_en=has_src1(spec)
        )
        shas[ver] = tmp.sha(ver)
    op = dve_ops.DveOp(name, spec, subdim=False, uops_sha=shas)
    dve_ops.OPS.append(op)
    dve_ops._SUB_OPCODE_FOR_NAME[name] = row
    dve_ops.CUSTOM_DVE_SPECS[name] = spec
    return op


def _emit_head_prep(nc, tc, pools, aps, h):
    """Loads + transposes for head h. Returns (qT, kT, va)."""
    (ld_pool, tq_pool, v_pool, pt_pool, _out_pool, _small_pool,
     ps_t, ps_s, _ps_o, ident, dconst) = pools
    q, k, v, _out = aps

    # Natural-layout loads with fp32->bf16 cast during DMA (SWDGE).
    qn = ld_pool.tile([128, S], BF16, tag="qn")
    kn = ld_pool.tile([128, S], BF16, tag="kn")
    nc.gpsimd.dma_start(
        out=qn[:].rearrange("p (sb d) -> p sb d", d=D),
        in_=q[h].rearrange("(sb p) d -> p sb d", p=128),
    )
    nc.gpsimd.dma_start(
        out=kn[:].rearrange("p (sb d) -> p sb d", d=D),
        in_=k[h].rearrange("(sb p) d -> p sb d", p=128),
    )

    # V augmented with a ones column: [128, NB*(D+1)] bf16.
    va = v_pool.tile([128, NB * (D + 1)], BF16, tag="va")
    va3 = va[:].rearrange("p (ib e) -> p ib e", e=D + 1)
    nc.gpsimd.dma_start(
        out=va3[:, :, 0:D],
        in_=v[h].rearrange("(ib p) d -> p ib d", p=128),
    )
    nc.gpsimd.memset(va3[:, :, D : D + 1], 1.0)

    # Transposes as normal-mode matmuls (block stationary, identity moving):
    # out[d, s] = blk[s, d].T @ I. These pipeline like regular matmuls.
    qT = tq_pool.tile([128, S], BF16, tag="qT")
    kT = tq_pool.tile([128, S], BF16, tag="kT")
    for src, dst in ((qn, qT), (kn, kT)):
        for half in range(2):
            pth = ps_t.tile([128, 512], FP32)
            for g in range(4):
                sb = half * 4 + g
                nc.tensor.matmul(
                    pth[:, g * 128 : (g + 1) * 128],
                    src[:, sb * 128 : (sb + 1) * 128],
                    ident[:],
                    start=True,
                    stop=True,
                )
            nc.vector.tensor_copy(dst[:, half * 512 : (half + 1) * 512], pth[:])

    return qT, kT, va


def _emit_qk_exp(nc, pools, exp_op, qT, kT, ib):
    """One i-block of QK^T + exp; returns the P^T tile."""
    (_ld, _tq, _v, pt_pool, _out, _small, _ps_t, ps_s, _ps_o, _id, dconst) = pools
    ps = ps_s.tile([128, S], FP32)
    for jh in range(2):
        nc.tensor.matmul(
            ps[:, jh * 512 : (jh + 1) * 512],
            kT[:, ib * 128 : (ib + 1) * 128],
            qT[:, jh * 512 : (jh + 1) * 512],
            start=True,
            stop=True,
        )
    ptile = pt_pool.tile([128, S], BF16, tag=f"pt{ib}")
    if ib < NB - DVE_EXP_IBS:
        nc.scalar.activation(ptile[:], ps[:], AF.Exp, scale=1.0 / D)
    else:
        nc.vector._custom_dve(
            exp_op, out=ptile[:], in0=ps[:], in1=dconst[:],
            s0=EA / D, s1=EB, imm2=EC,
        )
    return ptile


def _emit_pv_norm(nc, pools, ptiles, va, ob, jb):
    """One j-block of PV + normalize into ob."""
    (_ld, _tq, _v, _pt, _out, small_pool, _ps_t, _ps_s, ps_o, _id, _dc) = pools
    va3 = va[:].rearrange("p (ib e) -> p ib e", e=D + 1)
    po = ps_o.tile([128, D + 1], FP32)
    for ib in range(NB):
        nc.tensor.matmul(
            po[:],
            ptiles[ib][:, jb * 128 : (jb + 1) * 128],
            va3[:, ib, :],
            start=(ib == 0),
            stop=(ib == NB - 1),
        )
    rec = small_pool.tile([128, 1], FP32, tag="rec")
    nc.vector.reciprocal(rec[:], po[:, D : D + 1])
    nc.vector.tensor_scalar_mul(
        ob[:, jb * 128 : (jb + 1) * 128], po[:, 0:D], rec[:]
    )


def _emit_store(nc, pools, aps, h, ob):
    out = aps[3]
    nc.scalar.dma_start(
        out=out[h].rearrange("(jb p) d -> p jb d", p=128),
        in_=ob[:].rearrange("p (jb d) -> p jb d", d=D),
    )


def build_bass():
    exp_op = _register_exp_op()
    nc = bacc.Bacc("TRN2", target_bir_lowering=False, debug=False)
    q = nc.dram_tensor("q", [H, S, D], FP32, kind="ExternalInput").ap()
    k = nc.dram_tensor("k", [H, S, D], FP32, kind="ExternalInput").ap()
    v = nc.dram_tensor("v", [H, S, D], FP32, kind="ExternalInput").ap()
    out = nc.dram_tensor("out", [H, S, D], FP32, kind="ExternalOutput").ap()
    aps = (q, k, v, out)

    with ExitStack() as ctx:
        tc = ctx.enter_context(tile.TileContext(nc))
        const_pool = ctx.enter_context(tc.tile_pool(name="const", bufs=1))
        ident = const_pool.tile([128, 128], BF16)
        make_identity(nc, ident[:])
        dconst = const_pool.tile([128, S], FP32)
        nc.gpsimd.memset(dconst[:], ES / D)

        ld_pool = ctx.enter_context(tc.tile_pool(name="loads", bufs=2))
        tq_pool = ctx.enter_context(tc.tile_pool(name="qkT", bufs=2))
        v_pool = ctx.enter_context(tc.tile_pool(name="vaug", bufs=2))
        pt_pool = ctx.enter_context(tc.tile_pool(name="pT", bufs=2))
        out_pool = ctx.enter_context(tc.tile_pool(name="outs", bufs=2))
        small_pool = ctx.enter_context(tc.tile_pool(name="small", bufs=4))
        ps_t = ctx.enter_context(tc.tile_pool(name="ps_t", bufs=2, space="PSUM"))
        ps_s = ctx.enter_context(tc.tile_pool(name="ps_s", bufs=2, space="PSUM"))
        ps_o = ctx.enter_context(tc.tile_pool(name="ps_o", bufs=2, space="PSUM"))
        pools = (ld_pool, tq_pool, v_pool, pt_pool, out_pool, small_pool,
                 ps_t, ps_s, ps_o, ident, dconst)

        # Software pipeline, block-interleaved: QK/exp of head h alternate
        # with PV/normalize of head h-1 so the PE always has ready work.
        out_pool = pools[4]
        prev = None  # (ptiles, va) of head h-1
        for h in range(H + 1):
            if h < H:
                qT, kT, va = _emit_head_prep(nc, tc, pools, aps, h)
                ptiles = []
            if prev is not None:
                ob = out_pool.tile([128, S], FP32, tag="ob")
            for x in range(NB):
                if h < H:
                    ptiles.append(_emit_qk_exp(nc, pools, exp_op, qT, kT, x))
                if prev is not None:
                    _emit_pv_norm(nc, pools, prev[0], prev[1], ob, x)
            if prev is not None:
                _emit_store(nc, pools, aps, h - 1, ob)
            prev = (ptiles, va) if h < H else None
    nc.finalize()
    return nc


_NC_CACHE = None


def _get_nc():
    global _NC_CACHE
    if _NC_CACHE is None:
        _NC_CACHE = build_bass()
    return _NC_CACHE


def run_sharded(q, k, v, **kwargs):
    """q,k,v: full [8, 16, 1024, 128] fp32. Returns (results, BassKernelResults)."""
    B = q.shape[0]
    nc = _get_nc()
    in_maps = [
        {
            "q": np.ascontiguousarray(q[c], dtype=np.float32),
            "k": np.ascontiguousarray(k[c], dtype=np.float32),
            "v": np.ascontiguousarray(v[c], dtype=np.float32),
        }
        for c in range(B)
    ]
    res = run_bass_kernel_spmd(nc, in_maps, core_ids=list(range(B)), **kwargs)
    out = np.stack([res.results[c]["out"] for c in range(B)]).astype(np.float32)
    return out, res


def kernel(q, k, v):
    q = np.asarray(q)
    k = np.asarray(k)
    v = np.asarray(v)
    out, _ = run_sharded(q, k, v)
    return out


if __name__ == "__main__":
    rng = np.random.default_rng(0)
    q = rng.standard_normal((8, H, S, D), dtype=np.float32)
    k = rng.standard_normal((8, H, S, D), dtype=np.float32)
    v = rng.standard_normal((8, H, S, D), dtype=np.float32)
    o = kernel(q, k, v)
    print("out", o.shape, o.dtype, float(np.abs(o).mean()))
